# revision 4
# baseline (speedup 1.0000x reference)
"""DCN CrossNetwork kernel v2 — bf16 I/O + host-transposed (xT) layout.

Reference computation (B=16384, D=1024, L=4 layers):
    x0 = x
    for c in range(L):
        s = x_c @ w_c               # (B,) row-wise dot
        x_{c+1} = x0 * s[:,None] + b_c + x_c

Algebra (same as v1): x_c = x0 * a_c + r_c with per-row scalar a_c and
row-independent r_c = sum_{j<c} b_j, so
    a_{c+1} = a_c * (1 + U_c) + d_c,   U_c = x0 . w_c,  d_c = r_c . w_c
    out     = x0 * a_L + r_L
Only U = x0 @ W^T, a 4-step scan, and one fused multiply-add per element
are needed on device.

v2 key changes vs v1:
  * bf16 device I/O (tolerance is 2e-2; bf16 rounding costs ~3e-3):
    halves HBM traffic 16MB -> 8MB per core.
  * HOST pre-transposes x to xT [D, B] (and output back).  With d on
    partitions, U^T[c,b] = sum_d W[c,d] xT[d,b] is a plain accumulated
    matmul with the tiny W chunk as the stationary operand — the 128
    PE transposes/core and the 16 big PSUM->SBUF copies of v1 vanish.
  * a4 row orientation via tiny [4,128]/[128,1] PE transposes + a
    K=1 broadcast matmul (ones^T (x) a4row).
  * final out^T[d,b] = xT[d,b]*a4[b] + r4[d]: DVE tensor_tensor (x*a4B)
    + tensor_scalar (+r4 per-partition), both bf16 2x/4x modes.

Sharding: batch dim split across 8 cores (2048 rows each); weights/biases
replicated.  No collectives.
"""

import sys

for _p in ("/opt/trn_rl_repo",):
    if _p not in sys.path:
        sys.path.insert(0, _p)

import numpy as np

B, D, L = 16384, 1024, 4
N_CORES = 8
B_SHARD = B // N_CORES       # 2048 rows per core
P = 128                      # SBUF partitions
N_CHUNKS = D // P            # 8 d-chunks of 128

_BUILT = {}

DEFAULT_CFG = dict(
    n_strips=2,       # b-strips per core (pipeline granule)
    load_batch=8,     # d-chunks per load DMA (8 = one 2MB DMA per strip)
    store_batch=8,    # d-chunks per store DMA
    load_eng="sync",
    store_eng="gpsimd",
    const_eng="scalar",
    mm_n=512,         # N per U-matmul (PSUM bank limit for f32 out)
    ts_act=2,         # how many +r4 tensor_scalar ops per strip to put on ACT
    fuse_tt=1,        # 1: one TT per x-tile (a4B broadcast via 0-stride dim)
)


def build_bass(iters=1, mode="full", **cfg_over):
    import concourse.bass as bass
    import concourse.bacc as bacc
    import concourse.mybir as mybir
    import concourse.tile as tile

    cfg = {**DEFAULT_CFG, **cfg_over}
    f32 = mybir.dt.float32
    bf16 = mybir.dt.bfloat16
    Alu = mybir.AluOpType
    Act = mybir.ActivationFunctionType

    NS = cfg["n_strips"]
    SB = B_SHARD // NS            # strip width in b (1024 for NS=2)
    LB = cfg["load_batch"]
    STB = cfg["store_batch"]
    MMN = cfg["mm_n"]
    assert N_CHUNKS % LB == 0 and N_CHUNKS % STB == 0 and SB % MMN == 0

    nc = bacc.Bacc("TRN2", debug=False)

    # x partition-major: row s*128+p holds, for SBUF partition p of strip s,
    # all 8 d-chunks' b-rows concatenated: x_h[s*128+p, k*SB+b] =
    # xT[128k+p, strip-s b].  Each partition line is a single contiguous
    # 16KB HBM run -> max DMA descriptor efficiency.
    x_d = nc.dram_tensor(
        "x", [NS * P, N_CHUNKS * SB], bf16, kind="ExternalInput"
    ).ap()
    # wt[p, 4k+c] = W[c, 128k+p]
    wt_d = nc.dram_tensor("wt", [P, L * N_CHUNKS], bf16, kind="ExternalInput").ap()
    # r4p[p, k] = r4[128k+p]
    r4_d = nc.dram_tensor("r4", [P, N_CHUNKS], f32, kind="ExternalInput").ap()
    # d1[p, c] = d_c (replicated)
    d1_d = nc.dram_tensor("d1", [P, L], f32, kind="ExternalInput").ap()
    id4_d = nc.dram_tensor("id4", [L, L], f32, kind="ExternalInput").ap()
    id128_d = nc.dram_tensor("id128", [P, P], bf16, kind="ExternalInput").ap()
    ones_d = nc.dram_tensor("ones1", [1, P], bf16, kind="ExternalInput").ap()
    out_d = nc.dram_tensor(
        "out", [NS * P, N_CHUNKS * SB], bf16, kind="ExternalOutput"
    ).ap()

    def _engs(spec):
        m = {"sync": nc.sync, "scalar": nc.scalar, "gpsimd": nc.gpsimd}
        return [m[s] for s in spec.split(",")]

    load_engs = _engs(cfg["load_eng"])
    store_engs = _engs(cfg["store_eng"])
    const_eng = _engs(cfg["const_eng"])[0]

    with tile.TileContext(nc) as tc:
        from contextlib import ExitStack

        with ExitStack() as ctx:
            cpool = ctx.enter_context(tc.tile_pool(name="consts", bufs=1))
            xpool = ctx.enter_context(
                tc.tile_pool(name="x", bufs=(N_CHUNKS // LB) * NS + 1)
            )
            opool = ctx.enter_context(tc.tile_pool(name="o", bufs=4))
            utsb = ctx.enter_context(tc.tile_pool(name="utsb", bufs=2))
            ascr = ctx.enter_context(tc.tile_pool(name="ascr", bufs=3))
            a4sb = ctx.enter_context(tc.tile_pool(name="a4sb", bufs=2))
            a4Bsb = ctx.enter_context(tc.tile_pool(name="a4Bsb", bufs=2))
            utps = ctx.enter_context(
                tc.tile_pool(name="utps", bufs=1, space=bass.MemorySpace.PSUM)
            )
            stps = ctx.enter_context(
                tc.tile_pool(name="stps", bufs=2, space=bass.MemorySpace.PSUM)
            )
            a4ps = ctx.enter_context(
                tc.tile_pool(name="a4ps", bufs=1, space=bass.MemorySpace.PSUM)
            )
            a4Bps = ctx.enter_context(
                tc.tile_pool(name="a4Bps", bufs=2, space=bass.MemorySpace.PSUM)
            )

            # ---- constants (parallel queue so x loads start at t=0) ----
            id128_t = cpool.tile([P, P], bf16)
            const_eng.dma_start(id128_t[:], id128_d[:])
            id4_t = cpool.tile([L, L], f32)
            const_eng.dma_start(id4_t[:], id4_d[:])
            wt_t = cpool.tile([P, L * N_CHUNKS], bf16)
            const_eng.dma_start(wt_t[:], wt_d[:])
            r4_t = cpool.tile([P, N_CHUNKS], f32)
            const_eng.dma_start(r4_t[:], r4_d[:])
            d1_t = cpool.tile([P, L], f32)
            const_eng.dma_start(d1_t[:], d1_d[:])
            ones_t = cpool.tile([1, P], bf16)
            const_eng.dma_start(ones_t[:], ones_d[:])

            if mode in ("dma_only", "load_only", "store_only"):
                x_c = cpool.tile([P, LB * SB], bf16)
                nc.sync.dma_start(x_c[:], x_d[0:P, 0 : LB * SB])
                n_ld = NS * (N_CHUNKS // LB)
                for it in range(iters):
                    for g in range(n_ld):
                        s, gg = divmod(g, N_CHUNKS // LB)
                        c0 = gg * LB * SB
                        if mode != "store_only":
                            x_t = xpool.tile([P, LB * SB], bf16)
                            load_engs[g % len(load_engs)].dma_start(
                                x_t[:],
                                x_d[s * P : (s + 1) * P, c0 : c0 + LB * SB],
                            )
                        if mode != "load_only":
                            store_engs[g % len(store_engs)].dma_start(
                                out_d[s * P : (s + 1) * P, c0 : c0 + LB * SB],
                                x_c[:],
                            )
                nc.compile()
                return nc

            do_load = mode not in ("noload", "compute_only")
            do_store = mode not in ("nostore", "compute_only")
            pre_x = None
            if not do_load:
                pre_x = {}
                for s in range(NS):
                    for g in range(N_CHUNKS // LB):
                        x_t = cpool.tile([P, LB, SB], bf16)
                        c0 = g * LB * SB
                        nc.sync.dma_start(
                            x_t[:],
                            x_d[s * P : (s + 1) * P, c0 : c0 + LB * SB].rearrange(
                                "p (k b) -> p k b", b=SB
                            ),
                        )
                        pre_x[(s, g)] = x_t

            for it in range(iters):
                for s in range(NS):
                    # ---- loads: contiguous partition lines, LB chunks/DMA ----
                    x_ts = []
                    for g in range(N_CHUNKS // LB):
                        if not do_load:
                            x_ts.append(pre_x[(s, g)])
                            continue
                        x_t = xpool.tile([P, LB, SB], bf16)
                        c0 = g * LB * SB
                        load_engs[(s * (N_CHUNKS // LB) + g) % len(load_engs)].dma_start(
                            x_t[:],
                            x_d[s * P : (s + 1) * P, c0 : c0 + LB * SB].rearrange(
                                "p (k b) -> p k b", b=SB
                            ),
                        )
                        x_ts.append(x_t)

                    def xs(k):
                        return x_ts[k // LB][:, k % LB, :]

                    # ---- U^T[c,b] accumulation: 8 chunks x (SB/MMN) cols ----
                    ut_p = utps.tile([L, SB], f32)
                    for k in range(N_CHUNKS):
                        for h in range(SB // MMN):
                            nc.tensor.matmul(
                                ut_p[:, h * MMN : (h + 1) * MMN],
                                wt_t[:, L * k : L * (k + 1)],   # lhsT [K=128d, M=4]
                                xs(k)[:, h * MMN : (h + 1) * MMN],
                                start=(k == 0),
                                stop=(k == N_CHUNKS - 1),
                            )
                    # u' = 1 + U^T  (fused into PSUM->SBUF copy, stays f32)
                    ut_s = utsb.tile([L, SB], f32)
                    nc.scalar.activation(ut_s[:], ut_p[:], Act.Copy, bias=1.0)

                    # ---- a4 per 128-wide b-block: transpose + scan + transpose
                    a4row_p = a4ps.tile([1, SB], bf16)
                    for j in range(SB // P):
                        st_p = stps.tile([P, L], f32)
                        nc.tensor.transpose(
                            st_p[:], ut_s[:, j * P : (j + 1) * P], id4_t[:]
                        )
                        a_t = ascr.tile([P, L], bf16)
                        nc.vector.tensor_tensor_scan(
                            a_t[:], st_p[:], d1_t[:], initial=1.0,
                            op0=Alu.mult, op1=Alu.add,
                        )
                        nc.tensor.transpose(
                            a4row_p[:, j * P : (j + 1) * P],
                            a_t[:, L - 1 : L],
                            id128_t[:],
                        )
                    a4row_s = a4sb.tile([1, SB], bf16)
                    nc.scalar.copy(a4row_s[:], a4row_p[:])

                    # ---- broadcast a4 row across 128 partitions (K=1 matmul)
                    # half-at-a-time so the PSUM scratch is one bank
                    a4B_s = a4Bsb.tile([P, SB], bf16)
                    for h in range(SB // MMN):
                        a4B_p = a4Bps.tile([P, MMN], f32)
                        nc.tensor.matmul(
                            a4B_p[:],
                            ones_t[:],                       # lhsT [K=1, M=128]
                            a4row_s[:, h * MMN : (h + 1) * MMN],
                            start=True,
                            stop=True,
                        )
                        nc.scalar.copy(a4B_s[:, h * MMN : (h + 1) * MMN], a4B_p[:])

                    # ---- out^T = xT * a4B + r4 (per-partition) ; store ----
                    for g in range(N_CHUNKS // STB):
                        o_t = opool.tile([P, STB, SB], bf16)
                        if cfg["fuse_tt"]:
                            # one TT per x-tile: a4B re-read per chunk via a
                            # 0-stride broadcast dim
                            assert STB == LB
                            a4bc = (
                                a4B_s[:]
                                .rearrange("p (u b) -> p u b", u=1)
                                .broadcast_to((P, STB, SB))
                            )
                            nc.vector.tensor_tensor(
                                o_t[:], x_ts[g][:], a4bc, op=Alu.mult
                            )
                        for j in range(STB):
                            k = g * STB + j
                            if not cfg["fuse_tt"]:
                                nc.vector.tensor_tensor(
                                    o_t[:, j, :], xs(k), a4B_s[:], op=Alu.mult
                                )
                            if (k % N_CHUNKS) < cfg["ts_act"]:
                                nc.scalar.add(
                                    o_t[:, j, :], o_t[:, j, :], r4_t[:, k : k + 1]
                                )
                            else:
                                nc.vector.tensor_scalar_add(
                                    o_t[:, j, :], o_t[:, j, :], r4_t[:, k : k + 1]
                                )
                        if do_store:
                            c0 = g * STB * SB
                            store_engs[
                                (s * (N_CHUNKS // STB) + g) % len(store_engs)
                            ].dma_start(
                                out_d[
                                    s * P : (s + 1) * P, c0 : c0 + STB * SB
                                ].rearrange("p (k b) -> p k b", b=SB),
                                o_t[:],
                            )

    nc.compile()
    return nc


def host_constants(weights, biases):
    """Pack W^T, r4 per-chunk, d_c, identities (tiny O(L*D) host work)."""
    w = np.ascontiguousarray(np.asarray(weights, dtype=np.float32))
    b = np.ascontiguousarray(np.asarray(biases, dtype=np.float32))
    from ml_dtypes import bfloat16

    r = np.zeros(D, np.float32)
    d_vec = np.zeros(L, np.float32)
    for c in range(L):
        d_vec[c] = np.float32(r @ w[c])
        r = r + b[c]
    wt = np.transpose(w.reshape(L, N_CHUNKS, P), (2, 1, 0)).reshape(P, N_CHUNKS * L)
    wt = np.ascontiguousarray(wt.astype(bfloat16))
    r4p = np.ascontiguousarray(r.reshape(N_CHUNKS, P).T)          # [P, 8] f32
    d1_rep = np.ascontiguousarray(np.broadcast_to(d_vec, (P, L)))
    id4 = np.eye(L, dtype=np.float32)
    id128 = np.eye(P, dtype=bfloat16)
    ones1 = np.ones((1, P), dtype=bfloat16)
    return wt, r4p, d1_rep, id4, id128, ones1


def _get_built(key=None, **cfg):
    k = key or "default"
    if k not in _BUILT:
        _BUILT[k] = build_bass(**cfg)
    return _BUILT[k]


def pack_x(x):
    """x [B, D] f32 -> per-core [NS*128, 8*SB] bf16, partition-major:
    xb[c][s*128+p, k*SB+b] = x[c*2048 + s*SB + b, 128k + p]."""
    from ml_dtypes import bfloat16

    NS = DEFAULT_CFG["n_strips"]
    SB = B_SHARD // NS
    xb = (
        np.asarray(x, dtype=np.float32)
        .astype(bfloat16)
        .reshape(N_CORES, NS, SB, N_CHUNKS, P)
        .transpose(0, 1, 4, 3, 2)           # [core, s, p, k, b]
        .reshape(N_CORES, NS * P, N_CHUNKS * SB)
    )
    return np.ascontiguousarray(xb)


def unpack_out(o):
    """per-core [NS*128, 8*SB] bf16 -> [B, D] f32 (inverse of pack_x)."""
    NS = DEFAULT_CFG["n_strips"]
    SB = B_SHARD // NS
    return (
        np.asarray(o)
        .reshape(N_CORES, NS, P, N_CHUNKS, SB)
        .transpose(0, 1, 4, 3, 2)           # [core, s, b, k, p]
        .reshape(B, D)
        .astype(np.float32)
    )


def kernel(x, weights, biases, _trace=False):
    from concourse.bass_utils import run_bass_kernel_spmd

    x = np.asarray(x, dtype=np.float32)
    assert x.shape == (B, D), x.shape
    wt, r4p, d1_rep, id4, id128, ones1 = host_constants(weights, biases)

    xb = pack_x(x)

    nc = _get_built()
    in_maps = []
    for c in range(N_CORES):
        in_maps.append(
            {
                "x": xb[c],
                "wt": wt,
                "r4": r4p,
                "d1": d1_rep,
                "id4": id4,
                "id128": id128,
                "ones1": ones1,
            }
        )
    res = run_bass_kernel_spmd(nc, in_maps, list(range(N_CORES)), trace=_trace)
    o = np.stack([res.results[c]["out"] for c in range(N_CORES)], axis=0)
    out = unpack_out(o)
    if _trace:
        kernel.last_results = res
    return out
